# revision 71
# baseline (speedup 1.0000x reference)
"""Trainium2 Bass kernel for the attention-LSTM decoder (restructured).

Computation (all T positions share (h0, c0); see reference):
  h0 = tanh(eh @ bridge_hW.T);  c0 = tanh(ec @ bridge_cW.T)
  energy = tanh(enc @ key_W.T + h0 @ query_W.T);  scores = energy . eW
  alphas = softmax(mask(scores));  ctx = alphas @ enc
  gates = emb[tok] @ W_ih[:,:E].T + [ctx @ W_ih[:,E:].T + h0 @ W_hh.T + b]
  c = sig(f)*c0 + sig(i)*tanh(g);  h = sig(o)*tanh(c)
  out = emb[tok] @ preW[:,:E].T + h @ preW[:,E:E+H].T + ctx @ preW[:,E+H:].T

Sharding: data-parallel over batch B=128 across 8 cores (NB=16 each).

Key structure vs the naive version:
 - keyW projection loops (m,k) outer / batch inner with a k-major host
   layout so each stationary is loaded once and streams N=512 (2 batches).
 - scores land in a single [16,S] PSUM tile via diag-expanded energy_W
   stationaries; softmax runs once on [16,S] rows (exp uses accum_out).
 - ctx for all batches accumulates into one [16,2H] PSUM tile via
   masked-diagonal alphasT stationaries.
 - gate consts / out consts computed batch-major [16,4H]/[16,H] with
   N=512 matmuls, then PE-transposed to the per-partition layouts B needs.
 - phase B: token embeddings gathered in fp8, gates = one fp8 DoubleRow
   matmul per (hs,gate); bias applied on DVE (col-pair broadcast APs);
   activations batched as [128,1536] sigmoid + [128,512] tanh.
 - output projection accumulates oc + emb(fp8 DR) + h(fp16) in PSUM and
   DMAs straight from PSUM to DRAM.
"""

import numpy as np
import ml_dtypes
from contextlib import ExitStack

import concourse.bass as bass
import concourse.mybir as mybir
import concourse.tile as tile
from concourse import bacc
from concourse.bass_utils import run_bass_kernel_spmd
from concourse.masks import make_identity

FP32 = mybir.dt.float32
F16 = mybir.dt.float16
F8 = mybir.dt.float8e4
I32 = mybir.dt.int32
AF = mybir.ActivationFunctionType
OP = mybir.AluOpType
AX = mybir.AxisListType
PM = mybir.MatmulPerfMode

P = 128
H = 512
E = 256
TWOH = 1024
FOURH = 2048
S = 256
T = 256
V = 10000
N_CORES = 8
B_FULL = 128
NB = 16
NTOK = NB * T          # 4096
NTT = NTOK // 512      # 8 token tiles (512 tokens = 2 batches each)


def _load_chunked(nc, dst_tile, src_dram, k_chunks, n):
    """DRAM [k_chunks*128, n] -> SBUF [128, k_chunks*n] (chunk-major)."""
    src = src_dram[:].rearrange("(k p) n -> p k n", p=P)
    dst = dst_tile[:].rearrange("p (k n) -> p k n", k=k_chunks)
    nc.sync.dma_start(out=dst, in_=src)


def _colpair(t, col0, rep):
    """AP reading cols [col0, col0+1] of tile t, each broadcast rep times."""
    ap = t[:]
    return bass.AP(ap.tensor, ap.offset + col0, [ap.ap[0], [1, 2], [0, rep]])


def _diag_out(t, col0):
    """AP writing 16 cols of tile t at col0 + 17*j (block-diagonal)."""
    ap = t[:]
    return bass.AP(ap.tensor, ap.offset + col0, [ap.ap[0], [17, 16]])


def build_kernel(nc, debug=False):
    dt = lambda name, shape, dtype=FP32: nc.dram_tensor(
        name, shape, dtype, kind="ExternalInput")

    encT_d = dt("encTkm", [P, 8 * NB * S], F16)     # [p,(k b s)] k-major
    enc_d = dt("enc", [NB * S, TWOH], F16)          # S-major per batch
    # bridge path in fp16 (fp32 matmuls are 4 cycles/row on PE)
    ehT_d = dt("ehT", [TWOH, NB], F16)
    ecT_d = dt("ecT", [TWOH, NB], F16)
    idx_d = dt("idx", [P, NTOK // P], I32)
    mask_d = dt("mask", [NB, S])
    maskoff_d = dt("maskoff", [NB, S])
    emb_d = dt("emb", [V, E], F8)
    keyWT_d = dt("keyWT", [TWOH, H], F16)
    queryWT_d = dt("queryWT", [H, H], F16)
    eWd_d = dt("eWd", [P, 4 * NB * NB], F16)        # diag-expanded energy_W
    wih8_d = dt("wih8", [E, FOURH], F8)
    whhT_d = dt("whhT", [H, FOURH], F16)
    wcxT_d = dt("wcxT", [TWOH, FOURH], F16)
    biasg_d = dt("biasg", [1, FOURH], F16)
    bhWT_d = dt("bhWT", [TWOH, H], F16)
    bcWT_d = dt("bcWT", [TWOH, H], F16)
    hb_d = dt("hb", [P, 4])
    cb_d = dt("cb", [P, 4])
    preW8_d = dt("preW8", [E, H], F8)
    preWTh_d = dt("preWTh", [H, H], F16)
    preWTc_d = dt("preWTc", [TWOH, H], F16)
    out_d = nc.dram_tensor("out", [NTOK, H], F16, kind="ExternalOutput")
    oc_dram = nc.dram_tensor("oc_bounce", [NB, H], F16, kind="Internal")

    dbg = {}
    if debug:
        for name, shape, dty in [
            ("d_energy", [P, 4 * NB * S], F16), ("d_alpha", [NB, S], F16),
            ("d_ctx", [NB, TWOH], F16), ("d_gc", [NB, FOURH], F16),
            ("d_oc", [NB, H], F16), ("d_embT", [P, NTT * TWOH], F8),
            ("d_hT", [P, NTT * FOURH], F16),
        ]:
            dbg[name] = nc.dram_tensor(name, shape, dty, kind="ExternalOutput")

    with ExitStack() as ctx:
        tc = ctx.enter_context(tile.TileContext(nc))

        # ---------- constants ----------
        const = ctx.enter_context(tc.tile_pool(name="const", bufs=1))
        identity_h = const.tile([P, P], F16)
        make_identity(nc, identity_h[:])
        identity_8 = const.tile([P, P], F8)
        nc.vector.tensor_copy(identity_8[:], identity_h[:])
        ones16 = const.tile([1, NB], F16)
        nc.vector.memset(ones16[:], 1.0)

        idx_sb = const.tile([P, NTOK // P], I32)
        nc.sync.dma_start(out=idx_sb[:], in_=idx_d[:])
        mask_sb = const.tile([NB, S], FP32)
        nc.sync.dma_start(out=mask_sb[:], in_=mask_d[:])
        maskoff_sb = const.tile([NB, S], FP32)
        nc.sync.dma_start(out=maskoff_sb[:], in_=maskoff_d[:])
        eWd_sb = const.tile([P, 4 * NB * NB], F16)
        nc.sync.dma_start(out=eWd_sb[:], in_=eWd_d[:])
        biasg_sb = const.tile([1, FOURH], F16)
        nc.sync.dma_start(out=biasg_sb[:], in_=biasg_d[:])
        hb_sb = const.tile([P, 4], FP32)
        nc.sync.dma_start(out=hb_sb[:], in_=hb_d[:])
        cb_sb = const.tile([P, 4], FP32)
        nc.sync.dma_start(out=cb_sb[:], in_=cb_d[:])
        ehT_sb = const.tile([P, 8 * NB], F16)
        _load_chunked(nc, ehT_sb, ehT_d, 8, NB)
        ecT_sb = const.tile([P, 8 * NB], F16)
        _load_chunked(nc, ecT_sb, ecT_d, 8, NB)

        # ---------- token embedding gather (fp8), issued up front ----------
        gep = ctx.enter_context(tc.tile_pool(name="gep", bufs=1))
        ge_all = gep.tile([P, NTOK // P * E], F8)
        for j in range(NTOK // P):
            nc.gpsimd.indirect_dma_start(
                out=ge_all[:, j * E:(j + 1) * E], out_offset=None,
                in_=emb_d[:],
                in_offset=bass.IndirectOffsetOnAxis(
                    ap=idx_sb[:, j:j + 1], axis=0))

        # ---------- state ----------
        state = ctx.enter_context(tc.tile_pool(name="state", bufs=1))
        h0T_sb = state.tile([P, 4 * NB], FP32)
        c0T_sb = state.tile([P, 4 * NB], FP32)
        qprojT_sb = state.tile([P, 4 * NB], FP32)
        h0T_h = state.tile([P, 4 * NB], F16)
        c0T_h = state.tile([P, 4 * NB], F16)
        alphas_n = state.tile([NB, S], F16)
        amask = state.tile([P, 2 * S], F16)
        ctx_bm = state.tile([NB, TWOH], F16)
        ctxT_sb = state.tile([P, 8 * NB], F16)
        gc_bm = state.tile([NB, FOURH], F16)
        gcT_sb = state.tile([P, 16 * NB], FP32)
        oc_sb = state.tile([NB, H], F16)
        zsum = state.tile([NB, 1], FP32)
        rz = state.tile([NB, 1], FP32)
        nmx = state.tile([NB, 1], FP32)

        ebp = ctx.enter_context(tc.tile_pool(name="ebp", bufs=1))
        embT_all = ebp.tile([P, NTT * TWOH], F8)

        # ---------- setup: bridge h0/c0, qproj ----------
        kw_stack = ExitStack()
        ea = kw_stack.enter_context(tc.tile_pool(name="energy", bufs=1))
        energy_all = ea.tile([P, 4 * NB * S], F16)
        kwp = kw_stack.enter_context(tc.tile_pool(name="kw", bufs=1))
        keyWT_sb = kwp.tile([P, 8 * H], F16)
        encT_sb = kwp.tile([P, 8 * NB * S], F16)
        with tc.tile_pool(name="setup_w", bufs=1) as swp, \
             tc.tile_pool(name="setup_ps", bufs=2, space="PSUM") as sps:
            # bridge weights first in the DMA queue (setup-critical), then
            # the A1 inputs stream behind them
            bhWT_sb = swp.tile([P, 8 * H], F16, tag="bridgeh", name="bh")
            _load_chunked(nc, bhWT_sb, bhWT_d, 8, H)
            bcWT_sb = swp.tile([P, 8 * H], F16, tag="bridgec", name="bc")
            _load_chunked(nc, bcWT_sb, bcWT_d, 8, H)
            qWT_sb = swp.tile([P, 4 * H], F16, tag="bridgeq", name="qw")
            _load_chunked(nc, qWT_sb, queryWT_d, 4, H)
            _load_chunked(nc, keyWT_sb, keyWT_d, 8, H)
            for k in range(8):
                sl = slice(k * NB * S, (k + 1) * NB * S)
                nc.sync.dma_start(out=encT_sb[:, sl], in_=encT_d[:, sl])
            for m in range(4):
                ps = sps.tile([P, NB], FP32, tag="ps")
                for k in range(8):
                    nc.tensor.matmul(
                        ps[:], bhWT_sb[:, k * H + m * P:k * H + m * P + P],
                        ehT_sb[:, k * NB:(k + 1) * NB],
                        start=(k == 0), stop=(k == 7))
                nc.scalar.activation(h0T_sb[:, m * NB:(m + 1) * NB], ps[:],
                                     AF.Tanh, bias=hb_sb[:, m:m + 1])
            nc.vector.tensor_copy(h0T_h[:], h0T_sb[:])
            for m in range(4):
                ps = sps.tile([P, NB], FP32, tag="ps")
                for k in range(8):
                    nc.tensor.matmul(
                        ps[:], bcWT_sb[:, k * H + m * P:k * H + m * P + P],
                        ecT_sb[:, k * NB:(k + 1) * NB],
                        start=(k == 0), stop=(k == 7))
                nc.scalar.activation(c0T_sb[:, m * NB:(m + 1) * NB], ps[:],
                                     AF.Tanh, bias=cb_sb[:, m:m + 1])
            for m in range(4):
                ps = sps.tile([P, NB], FP32, tag="ps")
                for k in range(4):
                    nc.tensor.matmul(
                        ps[:], qWT_sb[:, k * H + m * P:k * H + m * P + P],
                        h0T_h[:, k * NB:(k + 1) * NB],
                        start=(k == 0), stop=(k == 3))
                nc.vector.tensor_copy(qprojT_sb[:, m * NB:(m + 1) * NB], ps[:])
            nc.vector.tensor_copy(c0T_h[:], c0T_sb[:])

        # ---------- A1: keyW projection -> energy (fp16) ----------
        with tc.tile_pool(name="ps_pk", bufs=1, space="PSUM") as ps_pk:
            pk_tiles = [ps_pk.tile([P, 2 * S], FP32, tag=f"pk{i}",
                                   name=f"pk{i}") for i in range(8)]
            # batches in 2 groups of 8; psum sets alternate so one group's
            # activations overlap the next group's matmuls
            for m in range(4):
                for bg in range(2):
                    pset = pk_tiles[((2 * m + bg) % 2) * 4:
                                    ((2 * m + bg) % 2) * 4 + 4]
                    for k in range(8):
                        stat = keyWT_sb[:, k * H + m * P:k * H + m * P + P]
                        for bpi in range(4):
                            bp = bg * 4 + bpi
                            nc.tensor.matmul(
                                pset[bpi][:], stat,
                                encT_sb[:, (k * NB + 2 * bp) * S:
                                        (k * NB + 2 * bp + 2) * S],
                                start=(k == 0), stop=(k == 7))
                    for bi in range(8):
                        b = bg * 8 + bi
                        nc.scalar.activation(
                            energy_all[:, (m * NB + b) * S:
                                       (m * NB + b + 1) * S],
                            pset[bi // 2][:, (bi % 2) * S:(bi % 2 + 1) * S],
                            AF.Tanh,
                            bias=qprojT_sb[:, m * NB + b:m * NB + b + 1])
        if debug:
            nc.sync.dma_start(out=dbg["d_energy"][:], in_=energy_all[:])


        # ---------- A2+A3: scores [16,S] + softmax ----------
        # embT for ALL ttiles is also transposed here: the PE transposes fill
        # the softmax latency gap and the copies run on otherwise-idle DVE.
        with tc.tile_pool(name="smx", bufs=1) as smx, \
             tc.tile_pool(name="ps_sc", bufs=1, space="PSUM") as ps_sc, \
             tc.tile_pool(name="ps_tp", bufs=2, space="PSUM") as ps_tp:
            scps = ps_sc.tile([NB, S], FP32)
            for m in range(4):
                for b in range(NB):
                    nc.tensor.matmul(
                        scps[:],
                        eWd_sb[:, (m * NB + b) * NB:(m * NB + b + 1) * NB],
                        energy_all[:, (m * NB + b) * S:(m * NB + b + 1) * S],
                        start=(m == 0 and b == 0), stop=(m == 3 and b == 15))
            sm = smx.tile([NB, S], FP32)
            nc.vector.tensor_tensor(out=sm[:], in0=scps[:], in1=mask_sb[:],
                                    op=OP.mult)
            nc.vector.tensor_tensor(out=sm[:], in0=sm[:], in1=maskoff_sb[:],
                                    op=OP.add)
            nc.vector.tensor_reduce(nmx[:], sm[:], AX.X, OP.max, negate=True)
            eu = smx.tile([NB, S], F16)
            nc.scalar.activation(eu[:], sm[:], AF.Exp, bias=nmx[:, 0:1],
                                 accum_out=zsum[:])
            nc.vector.reciprocal(rz[:], zsum[:])
            nc.vector.tensor_scalar_mul(alphas_n[:], eu[:], rz[:, 0:1])
            if debug:
                nc.sync.dma_start(out=dbg["d_alpha"][:], in_=alphas_n[:])
            # alphasT masked-diagonal expansion [128, 2*S]
            nc.vector.memset(amask[:], 0.0)
            tpa2 = ps_tp.tile([P, 2 * NB], F16, tag="tp")
            for c in range(2):
                nc.tensor.transpose(tpa2[:, c * NB:(c + 1) * NB],
                                    alphas_n[:, c * P:(c + 1) * P],
                                    identity_h[0:NB, 0:NB])
            for c in range(2):
                nc.vector.tensor_copy(_diag_out(amask, c * S),
                                      tpa2[:, c * NB:(c + 1) * NB])

        kw_stack.close()

        # phase-B weights: pool opened now that the A1 tiles are freed
        bw = ctx.enter_context(tc.tile_pool(name="bw", bufs=1))
        wih8_sb = bw.tile([P, 2 * FOURH], F8)
        _load_chunked(nc, wih8_sb, wih8_d, 2, FOURH)
        whh_sb = bw.tile([P, 4 * FOURH], F16)
        _load_chunked(nc, whh_sb, whhT_d, 4, FOURH)
        preW8_sb = bw.tile([P, 2 * H], F8)
        _load_chunked(nc, preW8_sb, preW8_d, 2, H)
        preWTh_sb = bw.tile([P, 4 * H], F16)
        _load_chunked(nc, preWTh_sb, preWTh_d, 4, H)
        pwc_sb = bw.tile([P, 8 * H], F16)
        _load_chunked(nc, pwc_sb, preWTc_d, 8, H)
        ocb_sb = bw.tile([P, NB * H], F16)

        # ---------- embT for all ttiles (PE transpose + Scalar copy) -----
        with tc.tile_pool(name="ps_tpE", bufs=4, space="PSUM") as ps_tpE:
            for tt in range(NTT):
                for j in range(4):
                    for e in range(2):
                        tp = ps_tpE.tile([P, 2 * P], F8, tag="tpB",
                                         name="tpB")
                        tpa = tp[:]
                        tp2 = bass.AP(tpa.tensor, tpa.offset,
                                      [tpa.ap[0], [2, P]])
                        nc.tensor.transpose(
                            tp2,
                            ge_all[:, (tt * 4 + j) * E + e * P:
                                   (tt * 4 + j) * E + (e + 1) * P],
                            identity_8[:])
                        nc.scalar.copy(
                            embT_all[:, tt * TWOH + e * H + j * P:
                                     tt * TWOH + e * H + (j + 1) * P], tp2)

        # ---------- overlapped A-tail (ctx/gc/oc per batch-half) + B -------
        # Half h covers batches h*8..h*8+8. Phase-B ttiles 0-3 run between
        # the halves so their Scalar/DVE work overlaps half 1's PE work.
        wcx_sb = bw.tile([P, 8 * FOURH], F16)
        _load_chunked(nc, wcx_sb, wcxT_d, 8, FOURH)
        wih8_v = wih8_sb[:].rearrange("p (k n) -> p k n", k=2)
        preW8_v = preW8_sb[:].rearrange("p (k n) -> p k n", k=2)
        nc.vector.memset(ctxT_sb[:], 0.0)
        with tc.tile_pool(name="encp", bufs=1) as encp, \
             tc.tile_pool(name="sgp", bufs=1) as sgp, \
             tc.tile_pool(name="hTp", bufs=2) as hTp, \
             tc.tile_pool(name="outp", bufs=3) as outp, \
             tc.tile_pool(name="ps_at", bufs=1, space="PSUM") as ps_at, \
             tc.tile_pool(name="ps_g", bufs=1, space="PSUM") as ps_g, \
             tc.tile_pool(name="ps_o", bufs=2, space="PSUM") as ps_o:
            enc_tiles = [encp.tile([P, 2 * TWOH], F16, tag=f"e{i}",
                                   name=f"enc{i}") for i in range(8)]

            def enc_dma(b):
                nc.sync.dma_start(
                    out=enc_tiles[b % 8][:].rearrange("p (c d) -> p c d",
                                                      c=2),
                    in_=enc_d[b * S:(b + 1) * S, :].rearrange(
                        "(c p) d -> p c d", p=P))

            def _strided_cols(t, h, blocks):
                ap = t[:]
                return bass.AP(ap.tensor, ap.offset + h * 8,
                               [ap.ap[0], [NB, blocks], [1, 8]])

            def a_tail_half(h):
                bs = list(range(h * 8, h * 8 + 8))
                for nh in range(2):
                    cp = ps_at.tile([NB, H], FP32, tag="acc", name="cp")
                    for bi, b in enumerate(bs):
                        et = enc_tiles[b % 8]
                        for sc in range(2):
                            nc.tensor.matmul(
                                cp[:],
                                amask[:, sc * S + b * NB:
                                      sc * S + (b + 1) * NB],
                                et[:, sc * TWOH + nh * H:
                                   sc * TWOH + (nh + 1) * H],
                                start=(bi == 0 and sc == 0),
                                stop=(bi == 7 and sc == 1))
                    nc.vector.tensor_copy(ctx_bm[:, nh * H:(nh + 1) * H],
                                          cp[:])
                # ctxT half-columns (full transposes; copy our half only)
                tpc = ps_at.tile([P, 8 * NB], F16, tag="tp", name="tpc")
                for kc in range(8):
                    nc.tensor.transpose(
                        tpc[:, kc * NB:(kc + 1) * NB],
                        ctx_bm[:, kc * P:(kc + 1) * P],
                        identity_h[0:NB, 0:NB])
                nc.vector.tensor_copy(
                    _strided_cols(ctxT_sb, h, 8),
                    bass.AP(tpc[:].tensor, tpc[:].offset + h * 8,
                            [tpc[:].ap[0], [NB, 8], [1, 8]]))
                # prefetch the other half's enc while B runs
                if h == 0:
                    for b in range(8, 16):
                        enc_dma(b)
                # gate consts: full-width stationaries; only rows of this
                # half are valid (other half's ctxT columns are zero/stale)
                for n in range(4):
                    sl = slice(n * H, (n + 1) * H)
                    gp = ps_at.tile([NB, H], FP32, tag="acc", name="gp")
                    for k in range(4):
                        nc.tensor.matmul(
                            gp[:], h0T_h[:, k * NB:(k + 1) * NB],
                            whh_sb[:, k * FOURH + n * H:
                                   k * FOURH + (n + 1) * H],
                            start=(k == 0), stop=False)
                    for kc in range(8):
                        nc.tensor.matmul(
                            gp[:], ctxT_sb[:, kc * NB:(kc + 1) * NB],
                            wcx_sb[:, kc * FOURH + n * H:
                                   kc * FOURH + (n + 1) * H],
                            start=False, stop=False)
                    nc.tensor.matmul(gp[:], ones16[0:1, :],
                                     biasg_sb[0:1, sl], start=False,
                                     stop=True)
                    nc.vector.tensor_copy(gc_bm[:, sl], gp[:])
                tpg = ps_at.tile([P, NB * NB], F16, tag="tp", name="tpg")
                for mg in range(16):
                    nc.tensor.transpose(
                        tpg[:, mg * NB:(mg + 1) * NB],
                        gc_bm[:, mg * P:(mg + 1) * P],
                        identity_h[0:NB, 0:NB])
                nc.vector.tensor_copy(
                    _strided_cols(gcT_sb, h, 16),
                    bass.AP(tpg[:].tensor, tpg[:].offset + h * 8,
                            [tpg[:].ap[0], [NB, 16], [1, 8]]))
                # out consts
                op_ = ps_at.tile([NB, H], FP32, tag="acc", name="op")
                for kc in range(8):
                    nc.tensor.matmul(op_[:],
                                     ctxT_sb[:, kc * NB:(kc + 1) * NB],
                                     pwc_sb[:, kc * H:(kc + 1) * H],
                                     start=(kc == 0), stop=(kc == 7))
                nc.vector.tensor_copy(oc_sb[:], op_[:])
                nc.sync.dma_start(out=oc_dram[:], in_=oc_sb[:])
                nc.sync.dma_start(
                    out=ocb_sb[:, h * 8 * H:(h * 8 + 8) * H].rearrange(
                        "p (b n) -> p b n", b=8),
                    in_=bass.AP(oc_dram[:].tensor, h * 8 * H,
                                [[0, P], [H, 8], [1, H]]))

            def gate_mm(hs, embT_v):
                # four separate 1-bank tiles [i, f, o, g] so each consumer
                # only waits on its own matmul (tile-granular deps)
                tiles = []
                for sl, g in enumerate((0, 1, 3, 2)):
                    mg = g * 4 + hs
                    gp = ps_g.tile([P, H], FP32, tag=f"g{sl}", name="gp")
                    nc.tensor.matmul(
                        gp[:], wih8_v[:, :, mg * P:(mg + 1) * P], embT_v,
                        start=True, stop=True, perf_mode=PM.DoubleRow)
                    tiles.append(gp)
                return tiles

            def outproj_mm(tt, embT_v, hts, tci):
                po = ps_o.tile([P, H], FP32, tag="po", name="po")
                nc.tensor.matmul(
                    po[:], embT_v[:, :, tci * P:(tci + 1) * P], preW8_v,
                    start=True, stop=False, perf_mode=PM.DoubleRow)
                for k in range(4):
                    nc.tensor.matmul(
                        po[:], hts[k][:, tci * P:(tci + 1) * P],
                        preWTh_sb[:, k * H:(k + 1) * H],
                        start=False, stop=(k == 3))
                return po

            def outproj_fin(tt, tci, po):
                # psum drain on Scalar, oc add on Pool, then DMA out
                b = 2 * tt + tci // 2
                oraw = outp.tile([P, H], F16, tag="oraw", name="oraw")
                nc.scalar.copy(oraw[:], po[:])
                o_t = outp.tile([P, H], F16, tag="o", name="o_t")
                nc.gpsimd.tensor_tensor(
                    out=o_t[:], in0=oraw[:],
                    in1=ocb_sb[:, b * H:(b + 1) * H], op=OP.add)
                nc.sync.dma_start(
                    out=out_d[tt * 512 + tci * P:tt * 512 + (tci + 1) * P,
                              :],
                    in_=o_t[:])

            prevB = [None]

            def b_ttile(tt):
                b0 = 2 * tt
                embT_v = embT_all[:, tt * TWOH:(tt + 1) * TWOH].rearrange(
                    "p (k t) -> p k t", k=2)
                hts = [hTp.tile([P, H], F16, tag=f"hT{i}", name="hts")
                       for i in range(4)]
                st = {}
                for hs in range(4):
                    gts = gate_mm(hs, embT_v)
                    po = (outproj_mm(tt - 1, prevB[0][0], prevB[0][1], hs)
                          if prevB[0] is not None else None)
                    sg = sgp.tile([P, 3 * H], F16, tag=f"sg{hs % 2}",
                                  name="sg")
                    for sl, g in enumerate((0, 1, 3)):
                        mg = g * 4 + hs
                        nc.vector.tensor_tensor(
                            out=sg[:, sl * H:(sl + 1) * H],
                            in0=gts[sl][:],
                            in1=_colpair(gcT_sb, mg * NB + b0, S),
                            op=OP.add)
                    ta_g = sgp.tile([P, H], F16, tag=f"tag{hs % 2}",
                                    name="ta_g")
                    for x in range(2):
                        mg = 2 * 4 + hs
                        nc.scalar.activation(
                            ta_g[:, x * S:(x + 1) * S],
                            gts[3][:, x * S:(x + 1) * S],
                            AF.Tanh,
                            bias=gcT_sb[:, mg * NB + b0 + x:
                                        mg * NB + b0 + x + 1])
                    if po is not None:
                        outproj_fin(tt - 1, hs, po)
                    st[hs] = [sg, ta_g]

                def tail(j):
                    sga, t1, t2 = st.pop(j)
                    cc = sgp.tile([P, H], F16, tag=f"cc{j % 2}", name="cc")
                    nc.gpsimd.tensor_tensor(out=cc[:], in0=t1[:],
                                            in1=t2[:], op=OP.add)
                    tanc = sgp.tile([P, H], F16, tag=f"tanc{j % 2}",
                                    name="tanc")
                    nc.scalar.activation(tanc[:], cc[:], AF.Tanh)
                    nc.vector.tensor_tensor(
                        out=hts[j][:], in0=sga[:, 2 * H:3 * H],
                        in1=tanc[:], op=OP.mult)

                for hs in range(4):
                    sg, ta_g = st[hs]
                    sga = sgp.tile([P, 3 * H], F16, tag=f"sga{hs % 2}",
                                   name="sga")
                    nc.scalar.activation(sga[:], sg[:], AF.Sigmoid)
                    t1 = sgp.tile([P, H], F16, tag=f"t1{hs % 2}",
                                  name="t1")
                    nc.vector.tensor_tensor(out=t1[:], in0=sga[:, 0:H],
                                            in1=ta_g[:], op=OP.mult)
                    t2 = sgp.tile([P, H], F16, tag=f"t2{hs % 2}",
                                  name="t2")
                    nc.gpsimd.tensor_tensor(
                        out=t2[:], in0=sga[:, H:2 * H],
                        in1=_colpair(c0T_h, hs * NB + b0, S), op=OP.mult)
                    st[hs] = [sga, t1, t2]
                    if hs > 0:
                        tail(hs - 1)
                tail(3)
                prevB[0] = (embT_v, hts)

            for b in range(8):
                enc_dma(b)
            a_tail_half(0)
            for tt in range(4):
                b_ttile(tt)
            a_tail_half(1)
            for tt in range(4, NTT):
                b_ttile(tt)
            for tci in range(4):
                po = outproj_mm(NTT - 1, prevB[0][0], prevB[0][1], tci)
                outproj_fin(NTT - 1, tci, po)
    return nc


# ---------------------------------------------------------------------------
# host side
# ---------------------------------------------------------------------------

def _chunkmajor(v, chunks, dtype=np.float32):
    return np.ascontiguousarray(v.reshape(chunks, P).T).astype(dtype)


def prep_inputs(inputs, n_cores=N_CORES):
    f32 = lambda x: np.asarray(x, dtype=np.float32)
    f16 = lambda x: np.ascontiguousarray(
        np.asarray(x, dtype=np.float32)).astype(np.float16)
    f8 = lambda x: np.ascontiguousarray(
        np.asarray(x, dtype=np.float32)).astype(ml_dtypes.float8_e4m3fn)
    tgt_seq = np.asarray(inputs["tgt_seq"]).astype(np.int32)
    enc = f32(inputs["encoder_output"])
    eh = f32(inputs["encoder_hidden"])[0]
    ec = f32(inputs["encoder_cell"])[0]
    src_pos = np.asarray(inputs["src_pos"])
    W_ih = f32(inputs["W_ih"])
    pre_W = f32(inputs["pre_W"])
    eW = f32(inputs["energy_W"])[0]

    eWd = np.zeros((P, 4, NB, NB), np.float16)
    for m in range(4):
        blk = eW[m * P:(m + 1) * P].astype(np.float16)
        for b in range(NB):
            eWd[:, m, b, b] = blk
    eWd = np.ascontiguousarray(eWd.reshape(P, 4 * NB * NB))

    shared = dict(
        emb=f8(inputs["emb"]),
        keyWT=f16(f32(inputs["key_W"]).T),
        queryWT=f16(f32(inputs["query_W"]).T),
        eWd=eWd,
        wih8=f8(W_ih[:, :E].T),
        whhT=f16(f32(inputs["W_hh"]).T),
        wcxT=f16(W_ih[:, E:].T),
        biasg=f16((f32(inputs["b_ih"]) + f32(inputs["b_hh"]))[None, :]),
        bhWT=f16(f32(inputs["bridge_hW"]).T),
        bcWT=f16(f32(inputs["bridge_cW"]).T),
        hb=_chunkmajor(f32(inputs["bridge_hb"]), 4),
        cb=_chunkmajor(f32(inputs["bridge_cb"]), 4),
        preW8=f8(pre_W[:, :E].T),
        preWTh=f16(pre_W[:, E:E + H].T),
        preWTc=f16(pre_W[:, E + H:].T),
    )

    in_maps = []
    for i in range(n_cores):
        sl = slice(i * NB, (i + 1) * NB)
        m = src_pos[sl, 0, :].astype(np.float32)
        enc16 = enc[sl].astype(np.float16)          # [NB, S, 2H]
        # k-major encT: [p, (k b s)]
        ekm = np.ascontiguousarray(
            enc16.transpose(2, 0, 1)                 # [2H, NB, S]
            .reshape(8, P, NB, S).transpose(1, 0, 2, 3)
            .reshape(P, 8 * NB * S))
        in_maps.append(dict(
            encTkm=ekm,
            enc=np.ascontiguousarray(enc16.reshape(NB * S, TWOH)),
            ehT=f16(eh[sl].T),
            ecT=f16(ec[sl].T),
            idx=np.ascontiguousarray(tgt_seq[sl].reshape(-1, P).T),
            mask=np.ascontiguousarray(m),
            maskoff=np.ascontiguousarray(-1e9 * (1.0 - m)),
            **shared,
        ))
    return in_maps, NB


_CACHED = {}


def _get_nc(key=0, debug=False):
    if key not in _CACHED:
        nc = bacc.Bacc("TRN2", target_bir_lowering=False, debug=False)
        build_kernel(nc, debug=debug)
        nc.compile()
        _CACHED[key] = nc
    return _CACHED[key]


def kernel(**inputs):
    in_maps, _ = prep_inputs(inputs, N_CORES)
    nc = _get_nc()
    res = run_bass_kernel_spmd(nc, in_maps, list(range(N_CORES)))
    B = np.asarray(inputs["tgt_seq"]).shape[0]
    out = np.empty((B, T, H), dtype=np.float32)
    for i in range(N_CORES):
        out[i * NB:(i + 1) * NB] = res.results[i]["out"].reshape(NB, T, H)
    return out
